# revision 30
# baseline (speedup 1.0000x reference)
"""Trainium2 Bass kernel for ColumnAttention:
    out = softmax(query @ x^T + bias) @ x        (per batch sample)

Shapes: x [64, 576, 1024] f32, query [576, 1024] f32, bias [576, 576] f32.
Data-parallel over batch across 8 NeuronCores (8 samples per core).

Per-core program; samples processed in PAIRS (pair key axis 2*576 = 1152 =
9*128 so every mm1 k-chunk has full 128 partitions).

  mm1 (fp8 e4m3, DoubleRow):
        scoresT[k, q] = sum_d x[k, d] * qT[d, q]
        lhsT = host-pretransposed x, rhs = qT; DoubleRow packs two d-chunks
        per matmul (K=256 effective) for 2x PE throughput. q split 288+288
        into the two banks of one 2-bank PSUM tile. One strided DVE add
        applies the bias on drain; ACT exp writes bf16 attnT.
        (mm1 in e4m3 costs ~1.2e-2 max rel err vs the 2e-2 budget --
        measured bit-exact against a host fp8 simulation.)
  mm2 (bf16):
        out[q, d] = attnT[k, q]^T @ x'[k, d'] per sample, where x' has a
        leading all-ones column. d' split into 3 passes (343+341+341 cols)
        so each pass fits one PSUM bank; pass 0's output column 0 is then
        exactly the softmax denominator -- no extra matmuls for it.
        DVE reciprocal + ACT/DVE scale drains produce bf16 output tiles.
        The q=512:576 tails of BOTH samples run as column-tiled concurrent
        matmuls (s0 -> out partitions 0:64, s1 -> 64:128).
  All HBM inputs ride the sync HWDGE ring in consumption order (per-ring
  FIFO gives head transfers full DMA bandwidth); outputs ride the gpsimd
  ring. ~10 wide warmup matmuls lift the PE HAM clock gate to 8/8 during
  the initial DMA wait. mm1 of pair p+1 interleaves into mm2 of pair p.
"""

import sys

if "/opt/trn_rl_repo" not in sys.path:
    sys.path.insert(0, "/opt/trn_rl_repo")

import numpy as np
import ml_dtypes
from contextlib import ExitStack

B, NQ, D = 64, 576, 1024
NCORES = 8
BPC = B // NCORES      # samples per core
NPAIR = BPC // 2       # sample pairs per core

P = 128
NKC = 2 * NQ // P      # 9 pair k-chunks
NDC = D // P           # 8 d chunks
KG = 3                 # xT DMA k-groups (384 pair-k each)
KGW = 2 * NQ // KG     # 384
QMAIN = 4              # full 128-row q chunks per sample (tail handled jointly)
DX = D + 1             # x natural width incl leading ones column
# mm2 d-passes over x' columns: (x'_offset, width). Pass 0 includes the
# ones column, so its out d-range is [0, 342); passes 1/2 pure x.
PASSES = [(0, 343, 0, 342), (343, 341, 342, 683), (684, 341, 683, 1024)]

_BUILD_CACHE = {}


def build_program():
    """Build + compile the per-core Bass program. Returns the Bacc object."""
    if "nc" in _BUILD_CACHE:
        return _BUILD_CACHE["nc"]

    import concourse.mybir as mybir
    import concourse.tile as tile
    from concourse import bacc

    bf16 = mybir.dt.bfloat16
    fp8 = mybir.dt.float8e4
    f32 = mybir.dt.float32
    AF = mybir.ActivationFunctionType
    DR = mybir.MatmulPerfMode.DoubleRow

    nc = bacc.Bacc(trn_type="TRN2", target_bir_lowering=False, debug=False)

    # x chunks 0..8 = pair-k natural; chunks 9/10 = chunk 4 with the s1/s0
    # rows zeroed, so the k-straddle runs as full-row matmuls (partial-row
    # LDWEIGHTS cannot use the background weight buffer and would expose
    # ~100ns per straddle matmul).
    xs = nc.dram_tensor("xs", [NPAIR, P, NKC + 2, DX], bf16, kind="ExternalInput")
    xsT = nc.dram_tensor("xsT", [NPAIR, P, KG, NDC, KGW], fp8, kind="ExternalInput")
    qT = nc.dram_tensor("qT", [P, NDC, NQ], fp8, kind="ExternalInput")
    bT = nc.dram_tensor("bT", [P, NKC, NQ], bf16, kind="ExternalInput")
    out = nc.dram_tensor("out", [BPC, NQ, D], bf16, kind="ExternalOutput")

    with tile.TileContext(nc) as tc, ExitStack() as ctx:
        statics = ctx.enter_context(tc.tile_pool(name="statics", bufs=1))
        xpool = ctx.enter_context(tc.tile_pool(name="xpool", bufs=2))
        xtpool = ctx.enter_context(tc.tile_pool(name="xtpool", bufs=2))
        scpool = ctx.enter_context(tc.tile_pool(name="scpool", bufs=3))
        atpool = ctx.enter_context(tc.tile_pool(name="atpool", bufs=2))
        ompool = ctx.enter_context(tc.tile_pool(name="ompool", bufs=2))
        otpool = ctx.enter_context(tc.tile_pool(name="otpool", bufs=2))
        rpool = ctx.enter_context(tc.tile_pool(name="rpool", bufs=4))
        junkpool = ctx.enter_context(tc.tile_pool(name="junk", bufs=1))
        # PSUM: 2*2 + 4*1 = 8 banks
        psAB = ctx.enter_context(tc.tile_pool(name="psAB", bufs=2, space="PSUM"))
        psO = ctx.enter_context(tc.tile_pool(name="psO", bufs=4, space="PSUM"))

        ones_sb = statics.tile([P, 1], bf16)
        nc.vector.memset(ones_sb, 1.0)
        garbage = junkpool.tile([P, 512], bf16)
        nc.vector.memset(garbage, 0.0)

        # ---- PE warmup: wide matmuls during the input DMA wait so the HAM
        # clock gate reaches 8/8 before the first real matmul (N=1 matmuls
        # leave the array ~idle and do NOT lift the gate). ----
        warm = psO.tile([P, 512], f32, tag="po")
        for _ in range(10):
            nc.tensor.matmul(warm[0:1, :], ones_sb, garbage, start=True, stop=True)
        junk = junkpool.tile([P, 1], f32)
        nc.vector.tensor_copy(junk[0:1, :], warm[0:1, 0:1])

        # ---- static params, all on the sync ring in consumption order.
        # qT and the first xT k-group are split per d-half so the first
        # matmul waits on ~0.5MB, not 2MB; bias follows, then bulk x. ----
        qT_sb = statics.tile([P, NDC, NQ], fp8)
        xT0_sb = xtpool.tile([P, KG, NDC, KGW], fp8, tag="xT")
        for h in range(2):
            nc.sync.dma_start(out=qT_sb[:, 4 * h:4 * h + 4, :],
                              in_=qT.ap()[:, 4 * h:4 * h + 4, :])
            nc.sync.dma_start(out=xT0_sb[:, 0, 4 * h:4 * h + 4, :],
                              in_=xsT.ap()[0, :, 0, 4 * h:4 * h + 4, :])
        bT_sb = statics.tile([P, NKC, NQ], bf16)
        nc.sync.dma_start(out=bT_sb[:, 0:3, :], in_=bT.ap()[:, 0:3, :])
        nc.sync.dma_start(out=bT_sb[:, 3:9, :], in_=bT.ap()[:, 3:9, :])

        def load_pair(pr, xT_sb=None, kg_start=0):
            """k-progressive xT then natural x, all on the sync ring."""
            if xT_sb is None:
                xT_sb = xtpool.tile([P, KG, NDC, KGW], fp8, tag="xT")
            for kg in range(kg_start, KG):
                nc.sync.dma_start(out=xT_sb[:, kg], in_=xsT.ap()[pr, :, kg])
            x_sb = xpool.tile([P, NKC + 2, DX], bf16, tag="x")
            nc.sync.dma_start(out=x_sb[:, 0:4, :], in_=xs.ap()[pr, :, 0:4, :])
            nc.sync.dma_start(out=x_sb[:, 9:11, :], in_=xs.ap()[pr, :, 9:11, :])
            nc.sync.dma_start(out=x_sb[:, 5:9, :], in_=xs.ap()[pr, :, 5:9, :])
            return x_sb, xT_sb

        def mm1_chunk(xT_sb, attnT, kc):
            """One pair k-chunk of scoresT + bias + exp. fp8 DoubleRow packs
            two d-chunks per matmul (K=256 effective). q halves 288+288 live
            in the two banks of one 2-bank PSUM tile."""
            kg, ks = kc // 3, (kc % 3) * P
            pa = psAB.tile([P, 2, 512], f32, tag="pa")
            for dr in range(NDC // 2):
                w = xT_sb[:, kg, 2 * dr:2 * dr + 2, ks:ks + P]
                st, sp = dr == 0, dr == NDC // 2 - 1
                nc.tensor.matmul(pa[:, 0, 0:288], w, qT_sb[:, 2 * dr:2 * dr + 2, 0:288],
                                 start=st, stop=sp, perf_mode=DR)
                nc.tensor.matmul(pa[:, 1, 0:288], w, qT_sb[:, 2 * dr:2 * dr + 2, 288:576],
                                 start=st, stop=sp, perf_mode=DR)
            sc = scpool.tile([P, 2, 288], f32, tag="sc")
            bv = bT_sb[:, kc, :].rearrange("p (h q) -> p h q", h=2)
            nc.vector.tensor_add(sc, pa[:, :, 0:288], bv)
            av = attnT[:, kc, :].rearrange("p (h q) -> p h q", h=2)
            nc.scalar.activation(av, sc, AF.Exp)

        # slot = (attnT k-chunk, x_sb chunk): the straddle chunk 4 reads the
        # per-sample zero-padded x copy (chunks 9/10) with FULL 128-row
        # weights -- the other sample's attn rows hit zeroed x rows.
        S0_SLOTS = [(c, c) for c in range(4)] + [(4, 9)]
        S1_SLOTS = [(4, 10)] + [(c, c) for c in range(5, 9)]

        def mm2_main(pr, s, qc, ps, x_sb, attnT, o_main, r_):
            """One (sample, 128-row q-chunk, d-pass) of out = attn @ x'.
            Pass 0's column 0 is the softmax denominator."""
            off, w_, d0, d1 = PASSES[ps]
            qb = qc * P
            slots = S0_SLOTS if s == 0 else S1_SLOTS
            po = psO.tile([P, 512], f32, tag="po")
            for j, (c, xc) in enumerate(slots):
                wt = attnT[:, c, qb:qb + P]
                st, sp = j == 0, j == len(slots) - 1
                nc.tensor.matmul(po[:, 0:w_], wt, x_sb[:, xc, off:off + w_],
                                 start=st, stop=sp)
            if ps == 0:
                nc.vector.reciprocal(r_[:, :], po[:, 0:1])
                nc.scalar.activation(o_main[:, qc, d0:d1], po[:, 1:w_], AF.Copy,
                                     scale=r_[:, :])
            elif ps == 1:
                nc.vector.tensor_scalar_mul(o_main[:, qc, d0:d1], po[:, 0:w_], r_[:, :])
            else:
                nc.scalar.activation(o_main[:, qc, d0:d1], po[:, 0:w_], AF.Copy,
                                     scale=r_[:, :])

        def mm2_tail(pr, ps, x_sb, attnT, o_tail, r_):
            """q 512:576 of BOTH samples, column-tiled: s0 -> out partitions
            0:64, s1 -> 64:128, alternating so the half-array matmuls run
            concurrently."""
            off, w_, d0, d1 = PASSES[ps]
            po = psO.tile([P, 512], f32, tag="po")
            na, nb = len(S0_SLOTS), len(S1_SLOTS)
            for j in range(na + nb):
                s, (c, xc) = (0, S0_SLOTS[j // 2]) if j % 2 == 0 else (1, S1_SLOTS[j // 2])
                wt = attnT[:, c, 512:576]
                st = j < 2
                sp = j >= na + nb - 2
                nc.tensor.matmul(po[64 * s:64 * s + 64, 0:w_], wt,
                                 x_sb[:, xc, off:off + w_],
                                 start=st, stop=sp)
            if ps == 0:
                nc.vector.reciprocal(r_[:, :], po[:, 0:1])
                nc.scalar.activation(o_tail[:, d0:d1], po[:, 1:w_], AF.Copy,
                                     scale=r_[:, :])
            elif ps == 1:
                nc.vector.tensor_scalar_mul(o_tail[:, d0:d1], po[:, 0:w_], r_[:, :])
            else:
                nc.scalar.activation(o_tail[:, d0:d1], po[:, 0:w_], AF.Copy,
                                     scale=r_[:, :])

        # ---- prologue: pair 0 loads + mm1 (kg0 already in flight above).
        # The early chunks are DMA-paced with ~1-2us stalls; a couple of
        # garbage matmuls after each keep the HAM activity window busy so
        # the PE clock stays at 8/8 through the fill phase. ----
        x_cur, xT_cur = load_pair(0, xT_sb=xT0_sb, kg_start=1)
        attnT_cur = atpool.tile([P, NKC, NQ], bf16, tag="attnT")
        for kc in range(NKC):
            mm1_chunk(xT_cur, attnT_cur, kc)
            if kc < 6:
                nc.tensor.matmul(warm[0:1, :], ones_sb, garbage,
                                 start=True, stop=True)
        nc.vector.tensor_copy(junk[0:1, :], warm[0:1, 1:2])

        # ---- steady: mm2(pair p) interleaved with mm1(pair p+1) ----
        for pr in range(NPAIR):
            if pr + 1 < NPAIR:
                x_nxt, xT_nxt = load_pair(pr + 1)
                attnT_nxt = atpool.tile([P, NKC, NQ], bf16, tag="attnT")
            else:
                x_nxt = xT_nxt = attnT_nxt = None

            o_mains = [ompool.tile([P, QMAIN, D], bf16, tag="om", name=f"om{pr}_{i}")
                       for i in range(2)]
            o_tail = otpool.tile([P, D], bf16, tag="ot")
            nunit = 0
            # mm1(p+1) chunks ride between mm2 units in PAIRS: each
            # fp8<->bf16 mode switch on the PE costs ~250ns, so fewer,
            # larger mm1 bursts beat one chunk per unit.
            CHUNK_AT = {3: (0, 1, 2), 7: (3, 4, 5), 11: (6, 7, 8)}

            def tick():
                nonlocal nunit
                if attnT_nxt is not None and nunit in CHUNK_AT:
                    for kc in CHUNK_AT[nunit]:
                        mm1_chunk(xT_nxt, attnT_nxt, kc)
                nunit += 1

            for qc in range(QMAIN):
                rs = [rpool.tile([P, 1], f32, tag="r", name=f"r{pr}_{qc}_{i}")
                      for i in range(2)]
                for ps in range(3):
                    for s in range(2):
                        mm2_main(pr, s, qc, ps, x_cur, attnT_cur, o_mains[s], rs[s])
                        if ps == 2:
                            # (s, qc) fully drained -> stream this chunk out
                            nc.gpsimd.dma_start(
                                out=out.ap()[2 * pr + s, qc * P:(qc + 1) * P, :],
                                in_=o_mains[s][:, qc, :])
                        tick()

            r_ = rpool.tile([P, 1], f32, tag="r")
            for ps in range(3):
                mm2_tail(pr, ps, x_cur, attnT_cur, o_tail, r_)
                tick()
            for s in range(2):
                nc.gpsimd.dma_start(out=out.ap()[2 * pr + s, 512:576, :],
                                    in_=o_tail[64 * s:64 * s + 64, :])

            x_cur, xT_cur, attnT_cur = x_nxt, xT_nxt, attnT_nxt

    nc.compile()
    _BUILD_CACHE["nc"] = nc
    return nc


def make_in_maps(x, query, bias):
    bf = ml_dtypes.bfloat16
    fp8 = ml_dtypes.float8_e4m3
    x_bf = x.astype(bf)
    x_f8 = x.astype(fp8)
    qTh = np.ascontiguousarray(
        query.T.astype(fp8).reshape(NDC, P, NQ).transpose(1, 0, 2))
    bTpair = np.concatenate([bias.T.astype(bf)] * 2, axis=0)       # [1152, 576]
    bTh = np.ascontiguousarray(bTpair.reshape(NKC, P, NQ).transpose(1, 0, 2))
    in_maps = []
    for c in range(NCORES):
        xp = x_bf[c * BPC:(c + 1) * BPC].reshape(NPAIR, 2 * NQ, D)
        # natural x with leading ones column, pair-k on partitions:
        # [pr, p, kc, 1+d]; chunks 9/10 = chunk 4 with s1/s0 rows zeroed.
        xh = xp.reshape(NPAIR, NKC, P, D).transpose(0, 2, 1, 3)
        ones = np.ones((NPAIR, P, NKC, 1), dtype=bf)
        xh = np.concatenate([ones, xh], axis=3)            # [pr, p, kc, DX]
        x4a = xh[:, :, 4:5, :].copy()
        x4a[:, 64:, 0, :] = 0                               # s0 view: zero s1 rows
        x4b = xh[:, :, 4:5, :].copy()
        x4b[:, :64, 0, :] = 0                               # s1 view: zero s0 rows
        xh = np.ascontiguousarray(np.concatenate([xh, x4a, x4b], axis=2))
        # transposed x (fp8, for mm1 weights): [pr, p(d in chunk), kg, dc, ks]
        xp8 = x_f8[c * BPC:(c + 1) * BPC].reshape(NPAIR, 2 * NQ, D)
        xTh = np.ascontiguousarray(
            xp8.reshape(NPAIR, KG, KGW, NDC, P).transpose(0, 4, 1, 3, 2))
        in_maps.append({"xs": xh, "xsT": xTh, "qT": qTh, "bT": bTh})
    return in_maps


def kernel(x, query, bias):
    from concourse.bass_utils import run_bass_kernel_spmd

    nc = build_program()
    in_maps = make_in_maps(np.asarray(x), np.asarray(query), np.asarray(bias))
    res = run_bass_kernel_spmd(nc, in_maps, core_ids=list(range(NCORES)))
    return np.concatenate(
        [r["out"].astype(np.float32) for r in res.results], axis=0)


if __name__ == "__main__":
    rng = np.random.default_rng(0)
    x = rng.standard_normal((B, NQ, D), dtype=np.float32)
    q = rng.standard_normal((NQ, D), dtype=np.float32) / 32.0
    bias = 0.01 * rng.standard_normal((NQ, NQ), dtype=np.float32)
    o = kernel(x, q, bias)
    print(o.shape, o.dtype)


# revision 31
# speedup vs baseline: 1.0057x; 1.0057x over previous
"""Trainium2 Bass kernel for ColumnAttention:
    out = softmax(query @ x^T + bias) @ x        (per batch sample)

Shapes: x [64, 576, 1024] f32, query [576, 1024] f32, bias [576, 576] f32.
Data-parallel over batch across 8 NeuronCores (8 samples per core).

Per-core program; samples processed in PAIRS (pair key axis 2*576 = 1152 =
9*128 so every mm1 k-chunk has full 128 partitions).

  mm1 (fp8 e4m3, DoubleRow):
        scoresT[k, q] = sum_d x[k, d] * qT[d, q]
        lhsT = host-pretransposed x, rhs = qT; DoubleRow packs two d-chunks
        per matmul (K=256 effective) for 2x PE throughput. q split 288+288
        into the two banks of one 2-bank PSUM tile. One strided DVE add
        applies the bias on drain; ACT exp writes bf16 attnT.
        (mm1 in e4m3 costs ~1.2e-2 max rel err vs the 2e-2 budget --
        measured bit-exact against a host fp8 simulation.)
  mm2 (bf16):
        out[q, d] = attnT[k, q]^T @ x'[k, d'] per sample, where x' has a
        leading all-ones column. d' split into 3 passes (343+341+341 cols)
        so each pass fits one PSUM bank; pass 0's output column 0 is then
        exactly the softmax denominator -- no extra matmuls for it.
        DVE reciprocal + ACT/DVE scale drains produce bf16 output tiles.
        The q=512:576 tails of BOTH samples run as column-tiled concurrent
        matmuls (s0 -> out partitions 0:64, s1 -> 64:128).
  All HBM inputs ride the sync HWDGE ring in consumption order (per-ring
  FIFO gives head transfers full DMA bandwidth); outputs ride the gpsimd
  ring. ~10 wide warmup matmuls lift the PE HAM clock gate to 8/8 during
  the initial DMA wait. mm1 of pair p+1 interleaves into mm2 of pair p.
"""

import sys

if "/opt/trn_rl_repo" not in sys.path:
    sys.path.insert(0, "/opt/trn_rl_repo")

import numpy as np
import ml_dtypes
from contextlib import ExitStack

B, NQ, D = 64, 576, 1024
NCORES = 8
BPC = B // NCORES      # samples per core
NPAIR = BPC // 2       # sample pairs per core

P = 128
NKC = 2 * NQ // P      # 9 pair k-chunks
NDC = D // P           # 8 d chunks
KG = 3                 # xT DMA k-groups (384 pair-k each)
KGW = 2 * NQ // KG     # 384
QMAIN = 4              # full 128-row q chunks per sample (tail handled jointly)
DX = D + 1             # x natural width incl leading ones column
# mm2 d-passes over x' columns: (x'_offset, width). Pass 0 includes the
# ones column, so its out d-range is [0, 342); passes 1/2 pure x.
PASSES = [(0, 343, 0, 342), (343, 341, 342, 683), (684, 341, 683, 1024)]

_BUILD_CACHE = {}


def build_program():
    """Build + compile the per-core Bass program. Returns the Bacc object."""
    if "nc" in _BUILD_CACHE:
        return _BUILD_CACHE["nc"]

    import concourse.mybir as mybir
    import concourse.tile as tile
    from concourse import bacc

    bf16 = mybir.dt.bfloat16
    fp8 = mybir.dt.float8e4
    f32 = mybir.dt.float32
    AF = mybir.ActivationFunctionType
    DR = mybir.MatmulPerfMode.DoubleRow

    nc = bacc.Bacc(trn_type="TRN2", target_bir_lowering=False, debug=False)

    # x chunks 0..8 = pair-k natural; chunks 9/10 = chunk 4 with the s1/s0
    # rows zeroed, so the k-straddle runs as full-row matmuls (partial-row
    # LDWEIGHTS cannot use the background weight buffer and would expose
    # ~100ns per straddle matmul).
    xs = nc.dram_tensor("xs", [NPAIR, P, NKC + 2, DX], bf16, kind="ExternalInput")
    xsT = nc.dram_tensor("xsT", [NPAIR, P, KG, NDC, KGW], fp8, kind="ExternalInput")
    qT = nc.dram_tensor("qT", [P, NDC, NQ], fp8, kind="ExternalInput")
    bT = nc.dram_tensor("bT", [P, NKC, NQ], bf16, kind="ExternalInput")
    out = nc.dram_tensor("out", [BPC, NQ, D], bf16, kind="ExternalOutput")

    with tile.TileContext(nc) as tc, ExitStack() as ctx:
        statics = ctx.enter_context(tc.tile_pool(name="statics", bufs=1))
        xpool = ctx.enter_context(tc.tile_pool(name="xpool", bufs=2))
        xtpool = ctx.enter_context(tc.tile_pool(name="xtpool", bufs=2))
        scpool = ctx.enter_context(tc.tile_pool(name="scpool", bufs=3))
        atpool = ctx.enter_context(tc.tile_pool(name="atpool", bufs=2))
        ompool = ctx.enter_context(tc.tile_pool(name="ompool", bufs=2))
        otpool = ctx.enter_context(tc.tile_pool(name="otpool", bufs=2))
        rpool = ctx.enter_context(tc.tile_pool(name="rpool", bufs=4))
        junkpool = ctx.enter_context(tc.tile_pool(name="junk", bufs=1))
        # PSUM: 2*2 + 4*1 = 8 banks
        psAB = ctx.enter_context(tc.tile_pool(name="psAB", bufs=2, space="PSUM"))
        psO = ctx.enter_context(tc.tile_pool(name="psO", bufs=4, space="PSUM"))

        ones_sb = statics.tile([P, 1], bf16)
        nc.vector.memset(ones_sb, 1.0)
        garbage = junkpool.tile([P, 512], bf16)
        nc.vector.memset(garbage, 0.0)

        # ---- PE warmup: wide matmuls during the input DMA wait so the HAM
        # clock gate reaches 8/8 before the first real matmul (N=1 matmuls
        # leave the array ~idle and do NOT lift the gate). ----
        warm = psO.tile([P, 512], f32, tag="po")
        for _ in range(10):
            nc.tensor.matmul(warm[0:1, :], ones_sb, garbage, start=True, stop=True)
        junk = junkpool.tile([P, 1], f32)
        nc.vector.tensor_copy(junk[0:1, :], warm[0:1, 0:1])

        # ---- static params, all on the sync ring in consumption order.
        # qT and the first xT k-group are split per d-half so the first
        # matmul waits on ~0.5MB, not 2MB; bias follows, then bulk x. ----
        qT_sb = statics.tile([P, NDC, NQ], fp8)
        xT0_sb = xtpool.tile([P, KG, NDC, KGW], fp8, tag="xT")
        for h in range(2):
            nc.sync.dma_start(out=qT_sb[:, 4 * h:4 * h + 4, :],
                              in_=qT.ap()[:, 4 * h:4 * h + 4, :])
            nc.sync.dma_start(out=xT0_sb[:, 0, 4 * h:4 * h + 4, :],
                              in_=xsT.ap()[0, :, 0, 4 * h:4 * h + 4, :])
        bT_sb = statics.tile([P, NKC, NQ], bf16)
        nc.sync.dma_start(out=bT_sb[:, 0:3, :], in_=bT.ap()[:, 0:3, :])
        nc.sync.dma_start(out=bT_sb[:, 3:9, :], in_=bT.ap()[:, 3:9, :])

        def load_pair(pr, xT_sb=None, kg_start=0):
            """k-progressive xT then natural x, all on the sync ring."""
            if xT_sb is None:
                xT_sb = xtpool.tile([P, KG, NDC, KGW], fp8, tag="xT")
            for kg in range(kg_start, KG):
                nc.sync.dma_start(out=xT_sb[:, kg], in_=xsT.ap()[pr, :, kg])
            x_sb = xpool.tile([P, NKC + 2, DX], bf16, tag="x")
            nc.sync.dma_start(out=x_sb[:, 0:4, :], in_=xs.ap()[pr, :, 0:4, :])
            nc.sync.dma_start(out=x_sb[:, 9:11, :], in_=xs.ap()[pr, :, 9:11, :])
            nc.sync.dma_start(out=x_sb[:, 5:9, :], in_=xs.ap()[pr, :, 5:9, :])
            return x_sb, xT_sb

        def mm1_chunk(xT_sb, attnT, kc):
            """One pair k-chunk of scoresT + bias + exp. fp8 DoubleRow packs
            two d-chunks per matmul (K=256 effective). q halves 288+288 live
            in the two banks of one 2-bank PSUM tile."""
            kg, ks = kc // 3, (kc % 3) * P
            pa = psAB.tile([P, 2, 512], f32, tag="pa")
            for dr in range(NDC // 2):
                w = xT_sb[:, kg, 2 * dr:2 * dr + 2, ks:ks + P]
                st, sp = dr == 0, dr == NDC // 2 - 1
                nc.tensor.matmul(pa[:, 0, 0:288], w, qT_sb[:, 2 * dr:2 * dr + 2, 0:288],
                                 start=st, stop=sp, perf_mode=DR)
                nc.tensor.matmul(pa[:, 1, 0:288], w, qT_sb[:, 2 * dr:2 * dr + 2, 288:576],
                                 start=st, stop=sp, perf_mode=DR)
            sc = scpool.tile([P, 2, 288], f32, tag="sc")
            bv = bT_sb[:, kc, :].rearrange("p (h q) -> p h q", h=2)
            nc.vector.tensor_add(sc, pa[:, :, 0:288], bv)
            av = attnT[:, kc, :].rearrange("p (h q) -> p h q", h=2)
            nc.scalar.activation(av, sc, AF.Exp)

        # slot = (attnT k-chunk, x_sb chunk): the straddle chunk 4 reads the
        # per-sample zero-padded x copy (chunks 9/10) with FULL 128-row
        # weights -- the other sample's attn rows hit zeroed x rows.
        S0_SLOTS = [(c, c) for c in range(4)] + [(4, 9)]
        S1_SLOTS = [(4, 10)] + [(c, c) for c in range(5, 9)]

        def mm2_main(pr, s, qc, ps, x_sb, attnT, o_main, r_):
            """One (sample, 128-row q-chunk, d-pass) of out = attn @ x'.
            Pass 0's column 0 is the softmax denominator."""
            off, w_, d0, d1 = PASSES[ps]
            qb = qc * P
            slots = S0_SLOTS if s == 0 else S1_SLOTS
            po = psO.tile([P, 512], f32, tag="po")
            for j, (c, xc) in enumerate(slots):
                wt = attnT[:, c, qb:qb + P]
                st, sp = j == 0, j == len(slots) - 1
                nc.tensor.matmul(po[:, 0:w_], wt, x_sb[:, xc, off:off + w_],
                                 start=st, stop=sp)
            if ps == 0:
                nc.vector.reciprocal(r_[:, :], po[:, 0:1])
                nc.scalar.activation(o_main[:, qc, d0:d1], po[:, 1:w_], AF.Copy,
                                     scale=r_[:, :])
            elif ps == 1:
                nc.vector.tensor_scalar_mul(o_main[:, qc, d0:d1], po[:, 0:w_], r_[:, :])
            else:
                nc.scalar.activation(o_main[:, qc, d0:d1], po[:, 0:w_], AF.Copy,
                                     scale=r_[:, :])

        def mm2_tail(pr, ps, x_sb, attnT, o_tail, r_):
            """q 512:576 of BOTH samples, column-tiled: s0 -> out partitions
            0:64, s1 -> 64:128, alternating so the half-array matmuls run
            concurrently."""
            off, w_, d0, d1 = PASSES[ps]
            po = psO.tile([P, 512], f32, tag="po")
            na, nb = len(S0_SLOTS), len(S1_SLOTS)
            for j in range(na + nb):
                s, (c, xc) = (0, S0_SLOTS[j // 2]) if j % 2 == 0 else (1, S1_SLOTS[j // 2])
                wt = attnT[:, c, 512:576]
                st = j < 2
                sp = j >= na + nb - 2
                nc.tensor.matmul(po[64 * s:64 * s + 64, 0:w_], wt,
                                 x_sb[:, xc, off:off + w_],
                                 start=st, stop=sp)
            if ps == 0:
                nc.vector.reciprocal(r_[:, :], po[:, 0:1])
                nc.scalar.activation(o_tail[:, d0:d1], po[:, 1:w_], AF.Copy,
                                     scale=r_[:, :])
            elif ps == 1:
                nc.vector.tensor_scalar_mul(o_tail[:, d0:d1], po[:, 0:w_], r_[:, :])
            else:
                nc.scalar.activation(o_tail[:, d0:d1], po[:, 0:w_], AF.Copy,
                                     scale=r_[:, :])

        # ---- prologue: pair 0 loads + mm1 (kg0 already in flight above).
        # The early chunks are DMA-paced with ~1-2us stalls; a couple of
        # garbage matmuls after each keep the HAM activity window busy so
        # the PE clock stays at 8/8 through the fill phase. ----
        x_cur, xT_cur = load_pair(0, xT_sb=xT0_sb, kg_start=1)
        attnT_cur = atpool.tile([P, NKC, NQ], bf16, tag="attnT")
        for kc in range(NKC):
            mm1_chunk(xT_cur, attnT_cur, kc)
            if kc < 6:
                for _ in range(2):
                    nc.tensor.matmul(warm[0:1, :], ones_sb, garbage,
                                     start=True, stop=True)
        nc.vector.tensor_copy(junk[0:1, :], warm[0:1, 1:2])

        # ---- steady: mm2(pair p) interleaved with mm1(pair p+1) ----
        for pr in range(NPAIR):
            if pr + 1 < NPAIR:
                x_nxt, xT_nxt = load_pair(pr + 1)
                attnT_nxt = atpool.tile([P, NKC, NQ], bf16, tag="attnT")
            else:
                x_nxt = xT_nxt = attnT_nxt = None

            o_mains = [ompool.tile([P, QMAIN, D], bf16, tag="om", name=f"om{pr}_{i}")
                       for i in range(2)]
            o_tail = otpool.tile([P, D], bf16, tag="ot")
            nunit = 0
            # mm1(p+1) chunks ride between mm2 units in PAIRS: each
            # fp8<->bf16 mode switch on the PE costs ~250ns, so fewer,
            # larger mm1 bursts beat one chunk per unit.
            CHUNK_AT = {3: (0, 1, 2), 7: (3, 4, 5), 11: (6, 7, 8)}

            def tick():
                nonlocal nunit
                if attnT_nxt is not None and nunit in CHUNK_AT:
                    for kc in CHUNK_AT[nunit]:
                        mm1_chunk(xT_nxt, attnT_nxt, kc)
                nunit += 1

            for qc in range(QMAIN):
                rs = [rpool.tile([P, 1], f32, tag="r", name=f"r{pr}_{qc}_{i}")
                      for i in range(2)]
                for ps in range(3):
                    for s in range(2):
                        mm2_main(pr, s, qc, ps, x_cur, attnT_cur, o_mains[s], rs[s])
                        if ps == 2:
                            # (s, qc) fully drained -> stream this chunk out
                            nc.gpsimd.dma_start(
                                out=out.ap()[2 * pr + s, qc * P:(qc + 1) * P, :],
                                in_=o_mains[s][:, qc, :])
                        tick()

            r_ = rpool.tile([P, 1], f32, tag="r")
            for ps in range(3):
                mm2_tail(pr, ps, x_cur, attnT_cur, o_tail, r_)
                tick()
            for s in range(2):
                nc.gpsimd.dma_start(out=out.ap()[2 * pr + s, 512:576, :],
                                    in_=o_tail[64 * s:64 * s + 64, :])

            x_cur, xT_cur, attnT_cur = x_nxt, xT_nxt, attnT_nxt

    nc.compile()
    _BUILD_CACHE["nc"] = nc
    return nc


def make_in_maps(x, query, bias):
    bf = ml_dtypes.bfloat16
    fp8 = ml_dtypes.float8_e4m3
    x_bf = x.astype(bf)
    x_f8 = x.astype(fp8)
    qTh = np.ascontiguousarray(
        query.T.astype(fp8).reshape(NDC, P, NQ).transpose(1, 0, 2))
    bTpair = np.concatenate([bias.T.astype(bf)] * 2, axis=0)       # [1152, 576]
    bTh = np.ascontiguousarray(bTpair.reshape(NKC, P, NQ).transpose(1, 0, 2))
    in_maps = []
    for c in range(NCORES):
        xp = x_bf[c * BPC:(c + 1) * BPC].reshape(NPAIR, 2 * NQ, D)
        # natural x with leading ones column, pair-k on partitions:
        # [pr, p, kc, 1+d]; chunks 9/10 = chunk 4 with s1/s0 rows zeroed.
        xh = xp.reshape(NPAIR, NKC, P, D).transpose(0, 2, 1, 3)
        ones = np.ones((NPAIR, P, NKC, 1), dtype=bf)
        xh = np.concatenate([ones, xh], axis=3)            # [pr, p, kc, DX]
        x4a = xh[:, :, 4:5, :].copy()
        x4a[:, 64:, 0, :] = 0                               # s0 view: zero s1 rows
        x4b = xh[:, :, 4:5, :].copy()
        x4b[:, :64, 0, :] = 0                               # s1 view: zero s0 rows
        xh = np.ascontiguousarray(np.concatenate([xh, x4a, x4b], axis=2))
        # transposed x (fp8, for mm1 weights): [pr, p(d in chunk), kg, dc, ks]
        xp8 = x_f8[c * BPC:(c + 1) * BPC].reshape(NPAIR, 2 * NQ, D)
        xTh = np.ascontiguousarray(
            xp8.reshape(NPAIR, KG, KGW, NDC, P).transpose(0, 4, 1, 3, 2))
        in_maps.append({"xs": xh, "xsT": xTh, "qT": qTh, "bT": bTh})
    return in_maps


def kernel(x, query, bias):
    from concourse.bass_utils import run_bass_kernel_spmd

    nc = build_program()
    in_maps = make_in_maps(np.asarray(x), np.asarray(query), np.asarray(bias))
    res = run_bass_kernel_spmd(nc, in_maps, core_ids=list(range(NCORES)))
    return np.concatenate(
        [r["out"].astype(np.float32) for r in res.results], axis=0)


if __name__ == "__main__":
    rng = np.random.default_rng(0)
    x = rng.standard_normal((B, NQ, D), dtype=np.float32)
    q = rng.standard_normal((NQ, D), dtype=np.float32) / 32.0
    bias = 0.01 * rng.standard_normal((NQ, NQ), dtype=np.float32)
    o = kernel(x, q, bias)
    print(o.shape, o.dtype)


# revision 32
# speedup vs baseline: 1.0154x; 1.0097x over previous
"""Trainium2 Bass kernel for ColumnAttention:
    out = softmax(query @ x^T + bias) @ x        (per batch sample)

Shapes: x [64, 576, 1024] f32, query [576, 1024] f32, bias [576, 576] f32.
Data-parallel over batch across 8 NeuronCores (8 samples per core).

Per-core program; samples processed in PAIRS (pair key axis 2*576 = 1152 =
9*128 so every mm1 k-chunk has full 128 partitions).

  mm1 (fp8 e4m3, DoubleRow):
        scoresT[k, q] = sum_d x[k, d] * qT[d, q]
        lhsT = host-pretransposed x, rhs = qT; DoubleRow packs two d-chunks
        per matmul (K=256 effective) for 2x PE throughput. q split 288+288
        into the two banks of one 2-bank PSUM tile. One strided DVE add
        applies the bias on drain; ACT exp writes bf16 attnT.
        (mm1 in e4m3 costs ~1.2e-2 max rel err vs the 2e-2 budget --
        measured bit-exact against a host fp8 simulation.)
  mm2 (bf16):
        out[q, d] = attnT[k, q]^T @ x'[k, d'] per sample, where x' has a
        leading all-ones column. d' split into 3 passes (343+341+341 cols)
        so each pass fits one PSUM bank; pass 0's output column 0 is then
        exactly the softmax denominator -- no extra matmuls for it.
        DVE reciprocal + ACT/DVE scale drains produce bf16 output tiles.
        The q=512:576 tails of BOTH samples run as column-tiled concurrent
        matmuls (s0 -> out partitions 0:64, s1 -> 64:128).
  All HBM inputs ride the sync HWDGE ring in consumption order (per-ring
  FIFO gives head transfers full DMA bandwidth); outputs ride the gpsimd
  ring. ~10 wide warmup matmuls lift the PE HAM clock gate to 8/8 during
  the initial DMA wait. mm1 of pair p+1 interleaves into mm2 of pair p.
"""

import sys

if "/opt/trn_rl_repo" not in sys.path:
    sys.path.insert(0, "/opt/trn_rl_repo")

import numpy as np
import ml_dtypes
from contextlib import ExitStack

B, NQ, D = 64, 576, 1024
NCORES = 8
BPC = B // NCORES      # samples per core
NPAIR = BPC // 2       # sample pairs per core

P = 128
NKC = 2 * NQ // P      # 9 pair k-chunks
NDC = D // P           # 8 d chunks
KG = 3                 # xT DMA k-groups (384 pair-k each)
KGW = 2 * NQ // KG     # 384
QMAIN = 4              # full 128-row q chunks per sample (tail handled jointly)
DX = D + 1             # x natural width incl leading ones column
# mm2 d-passes over x' columns: (x'_offset, width). Pass 0 includes the
# ones column, so its out d-range is [0, 342); passes 1/2 pure x.
PASSES = [(0, 343, 0, 342), (343, 341, 342, 683), (684, 341, 683, 1024)]

_BUILD_CACHE = {}


def build_program():
    """Build + compile the per-core Bass program. Returns the Bacc object."""
    if "nc" in _BUILD_CACHE:
        return _BUILD_CACHE["nc"]

    import concourse.mybir as mybir
    import concourse.tile as tile
    from concourse import bacc

    bf16 = mybir.dt.bfloat16
    fp8 = mybir.dt.float8e4
    f32 = mybir.dt.float32
    AF = mybir.ActivationFunctionType
    DR = mybir.MatmulPerfMode.DoubleRow

    nc = bacc.Bacc(trn_type="TRN2", target_bir_lowering=False, debug=False)

    # x chunks 0..8 = pair-k natural; chunks 9/10 = chunk 4 with the s1/s0
    # rows zeroed, so the k-straddle runs as full-row matmuls (partial-row
    # LDWEIGHTS cannot use the background weight buffer and would expose
    # ~100ns per straddle matmul).
    xs = nc.dram_tensor("xs", [NPAIR, P, NKC + 2, DX], bf16, kind="ExternalInput")
    xsT = nc.dram_tensor("xsT", [NPAIR, P, KG, NDC, KGW], fp8, kind="ExternalInput")
    qT = nc.dram_tensor("qT", [P, NDC, NQ], fp8, kind="ExternalInput")
    bT = nc.dram_tensor("bT", [P, NKC, NQ], bf16, kind="ExternalInput")
    out = nc.dram_tensor("out", [BPC, NQ, D], bf16, kind="ExternalOutput")

    with tile.TileContext(nc) as tc, ExitStack() as ctx:
        statics = ctx.enter_context(tc.tile_pool(name="statics", bufs=1))
        xpool = ctx.enter_context(tc.tile_pool(name="xpool", bufs=2))
        xtpool = ctx.enter_context(tc.tile_pool(name="xtpool", bufs=2))
        scpool = ctx.enter_context(tc.tile_pool(name="scpool", bufs=3))
        atpool = ctx.enter_context(tc.tile_pool(name="atpool", bufs=2))
        ompool = ctx.enter_context(tc.tile_pool(name="ompool", bufs=2))
        otpool = ctx.enter_context(tc.tile_pool(name="otpool", bufs=2))
        rpool = ctx.enter_context(tc.tile_pool(name="rpool", bufs=4))
        junkpool = ctx.enter_context(tc.tile_pool(name="junk", bufs=1))
        # PSUM: 2*2 + 4*1 = 8 banks
        psAB = ctx.enter_context(tc.tile_pool(name="psAB", bufs=2, space="PSUM"))
        psO = ctx.enter_context(tc.tile_pool(name="psO", bufs=4, space="PSUM"))

        ones_sb = statics.tile([P, 1], bf16)
        nc.vector.memset(ones_sb, 1.0)
        garbage = junkpool.tile([P, 512], bf16)
        nc.vector.memset(garbage, 0.0)

        # ---- PE warmup: wide matmuls during the input DMA wait so the HAM
        # clock gate reaches 8/8 before the first real matmul (N=1 matmuls
        # leave the array ~idle and do NOT lift the gate). ----
        warm = psO.tile([P, 512], f32, tag="po")
        for _ in range(9):
            nc.tensor.matmul(warm[0:1, :], ones_sb, garbage, start=True, stop=True)
        junk = junkpool.tile([P, 1], f32)
        nc.vector.tensor_copy(junk[0:1, :], warm[0:1, 0:1])

        # ---- static params, all on the sync ring in consumption order.
        # qT and the first xT k-group are split per d-half so the first
        # matmul waits on ~0.5MB, not 2MB; bias follows, then bulk x. ----
        qT_sb = statics.tile([P, NDC, NQ], fp8)
        xT0_sb = xtpool.tile([P, KG, NDC, KGW], fp8, tag="xT")
        for h in range(2):
            nc.sync.dma_start(out=qT_sb[:, 4 * h:4 * h + 4, :],
                              in_=qT.ap()[:, 4 * h:4 * h + 4, :])
            nc.sync.dma_start(out=xT0_sb[:, 0, 4 * h:4 * h + 4, :],
                              in_=xsT.ap()[0, :, 0, 4 * h:4 * h + 4, :])
        bT_sb = statics.tile([P, NKC, NQ], bf16)
        nc.sync.dma_start(out=bT_sb[:, 0:3, :], in_=bT.ap()[:, 0:3, :])
        nc.sync.dma_start(out=bT_sb[:, 3:9, :], in_=bT.ap()[:, 3:9, :])

        def load_pair(pr, xT_sb=None, kg_start=0):
            """k-progressive xT then natural x, all on the sync ring."""
            if xT_sb is None:
                xT_sb = xtpool.tile([P, KG, NDC, KGW], fp8, tag="xT")
            for kg in range(kg_start, KG):
                nc.sync.dma_start(out=xT_sb[:, kg], in_=xsT.ap()[pr, :, kg])
            x_sb = xpool.tile([P, NKC + 2, DX], bf16, tag="x")
            nc.sync.dma_start(out=x_sb[:, 0:4, :], in_=xs.ap()[pr, :, 0:4, :])
            nc.sync.dma_start(out=x_sb[:, 9:11, :], in_=xs.ap()[pr, :, 9:11, :])
            nc.sync.dma_start(out=x_sb[:, 5:9, :], in_=xs.ap()[pr, :, 5:9, :])
            return x_sb, xT_sb

        def mm1_chunk(xT_sb, attnT, kc):
            """One pair k-chunk of scoresT + bias + exp. fp8 DoubleRow packs
            two d-chunks per matmul (K=256 effective). q halves 288+288 live
            in the two banks of one 2-bank PSUM tile."""
            kg, ks = kc // 3, (kc % 3) * P
            pa = psAB.tile([P, 2, 512], f32, tag="pa")
            for dr in range(NDC // 2):
                w = xT_sb[:, kg, 2 * dr:2 * dr + 2, ks:ks + P]
                st, sp = dr == 0, dr == NDC // 2 - 1
                nc.tensor.matmul(pa[:, 0, 0:288], w, qT_sb[:, 2 * dr:2 * dr + 2, 0:288],
                                 start=st, stop=sp, perf_mode=DR)
                nc.tensor.matmul(pa[:, 1, 0:288], w, qT_sb[:, 2 * dr:2 * dr + 2, 288:576],
                                 start=st, stop=sp, perf_mode=DR)
            sc = scpool.tile([P, 2, 288], f32, tag="sc")
            bv = bT_sb[:, kc, :].rearrange("p (h q) -> p h q", h=2)
            nc.vector.tensor_add(sc, pa[:, :, 0:288], bv)
            av = attnT[:, kc, :].rearrange("p (h q) -> p h q", h=2)
            nc.scalar.activation(av, sc, AF.Exp)

        # slot = (attnT k-chunk, x_sb chunk): the straddle chunk 4 reads the
        # per-sample zero-padded x copy (chunks 9/10) with FULL 128-row
        # weights -- the other sample's attn rows hit zeroed x rows.
        S0_SLOTS = [(c, c) for c in range(4)] + [(4, 9)]
        S1_SLOTS = [(4, 10)] + [(c, c) for c in range(5, 9)]

        def mm2_main(pr, s, qc, ps, x_sb, attnT, o_main, r_):
            """One (sample, 128-row q-chunk, d-pass) of out = attn @ x'.
            Pass 0's column 0 is the softmax denominator."""
            off, w_, d0, d1 = PASSES[ps]
            qb = qc * P
            slots = S0_SLOTS if s == 0 else S1_SLOTS
            po = psO.tile([P, 512], f32, tag="po")
            for j, (c, xc) in enumerate(slots):
                wt = attnT[:, c, qb:qb + P]
                st, sp = j == 0, j == len(slots) - 1
                nc.tensor.matmul(po[:, 0:w_], wt, x_sb[:, xc, off:off + w_],
                                 start=st, stop=sp)
            if ps == 0:
                nc.vector.reciprocal(r_[:, :], po[:, 0:1])
                nc.scalar.activation(o_main[:, qc, d0:d1], po[:, 1:w_], AF.Copy,
                                     scale=r_[:, :])
            elif ps == 1:
                nc.vector.tensor_scalar_mul(o_main[:, qc, d0:d1], po[:, 0:w_], r_[:, :])
            else:
                nc.scalar.activation(o_main[:, qc, d0:d1], po[:, 0:w_], AF.Copy,
                                     scale=r_[:, :])

        def mm2_tail(pr, ps, x_sb, attnT, o_tail, r_):
            """q 512:576 of BOTH samples, column-tiled: s0 -> out partitions
            0:64, s1 -> 64:128, alternating so the half-array matmuls run
            concurrently."""
            off, w_, d0, d1 = PASSES[ps]
            po = psO.tile([P, 512], f32, tag="po")
            na, nb = len(S0_SLOTS), len(S1_SLOTS)
            for j in range(na + nb):
                s, (c, xc) = (0, S0_SLOTS[j // 2]) if j % 2 == 0 else (1, S1_SLOTS[j // 2])
                wt = attnT[:, c, 512:576]
                st = j < 2
                sp = j >= na + nb - 2
                nc.tensor.matmul(po[64 * s:64 * s + 64, 0:w_], wt,
                                 x_sb[:, xc, off:off + w_],
                                 start=st, stop=sp)
            if ps == 0:
                nc.vector.reciprocal(r_[:, :], po[:, 0:1])
                nc.scalar.activation(o_tail[:, d0:d1], po[:, 1:w_], AF.Copy,
                                     scale=r_[:, :])
            elif ps == 1:
                nc.vector.tensor_scalar_mul(o_tail[:, d0:d1], po[:, 0:w_], r_[:, :])
            else:
                nc.scalar.activation(o_tail[:, d0:d1], po[:, 0:w_], AF.Copy,
                                     scale=r_[:, :])

        # ---- prologue: pair 0 loads + mm1 (kg0 already in flight above).
        # The early chunks are DMA-paced with ~1-2us stalls; a couple of
        # garbage matmuls after each keep the HAM activity window busy so
        # the PE clock stays at 8/8 through the fill phase. ----
        x_cur, xT_cur = load_pair(0, xT_sb=xT0_sb, kg_start=1)
        attnT_cur = atpool.tile([P, NKC, NQ], bf16, tag="attnT")
        for kc in range(NKC):
            mm1_chunk(xT_cur, attnT_cur, kc)
            if kc < 6:
                for _ in range(2):
                    nc.tensor.matmul(warm[0:1, :], ones_sb, garbage,
                                     start=True, stop=True)
        nc.vector.tensor_copy(junk[0:1, :], warm[0:1, 1:2])

        # ---- steady: mm2(pair p) interleaved with mm1(pair p+1) ----
        for pr in range(NPAIR):
            if pr + 1 < NPAIR:
                x_nxt, xT_nxt = load_pair(pr + 1)
                attnT_nxt = atpool.tile([P, NKC, NQ], bf16, tag="attnT")
            else:
                x_nxt = xT_nxt = attnT_nxt = None

            o_mains = [ompool.tile([P, QMAIN, D], bf16, tag="om", name=f"om{pr}_{i}")
                       for i in range(2)]
            o_tail = otpool.tile([P, D], bf16, tag="ot")
            nunit = 0
            # mm1(p+1) chunks ride between mm2 units in PAIRS: each
            # fp8<->bf16 mode switch on the PE costs ~250ns, so fewer,
            # larger mm1 bursts beat one chunk per unit.
            CHUNK_AT = {3: (0, 1, 2), 7: (3, 4, 5), 11: (6, 7, 8)}

            def tick():
                nonlocal nunit
                if attnT_nxt is not None and nunit in CHUNK_AT:
                    for kc in CHUNK_AT[nunit]:
                        mm1_chunk(xT_nxt, attnT_nxt, kc)
                nunit += 1

            for qc in range(QMAIN):
                rs = [rpool.tile([P, 1], f32, tag="r", name=f"r{pr}_{qc}_{i}")
                      for i in range(2)]
                for ps in range(3):
                    for s in range(2):
                        mm2_main(pr, s, qc, ps, x_cur, attnT_cur, o_mains[s], rs[s])
                        if ps == 2:
                            # (s, qc) fully drained -> stream this chunk out
                            nc.gpsimd.dma_start(
                                out=out.ap()[2 * pr + s, qc * P:(qc + 1) * P, :],
                                in_=o_mains[s][:, qc, :])
                        tick()

            r_ = rpool.tile([P, 1], f32, tag="r")
            for ps in range(3):
                mm2_tail(pr, ps, x_cur, attnT_cur, o_tail, r_)
                tick()
            for s in range(2):
                nc.gpsimd.dma_start(out=out.ap()[2 * pr + s, 512:576, :],
                                    in_=o_tail[64 * s:64 * s + 64, :])

            x_cur, xT_cur, attnT_cur = x_nxt, xT_nxt, attnT_nxt

    nc.compile()
    _BUILD_CACHE["nc"] = nc
    return nc


def make_in_maps(x, query, bias):
    bf = ml_dtypes.bfloat16
    fp8 = ml_dtypes.float8_e4m3
    x_bf = x.astype(bf)
    x_f8 = x.astype(fp8)
    qTh = np.ascontiguousarray(
        query.T.astype(fp8).reshape(NDC, P, NQ).transpose(1, 0, 2))
    bTpair = np.concatenate([bias.T.astype(bf)] * 2, axis=0)       # [1152, 576]
    bTh = np.ascontiguousarray(bTpair.reshape(NKC, P, NQ).transpose(1, 0, 2))
    in_maps = []
    for c in range(NCORES):
        xp = x_bf[c * BPC:(c + 1) * BPC].reshape(NPAIR, 2 * NQ, D)
        # natural x with leading ones column, pair-k on partitions:
        # [pr, p, kc, 1+d]; chunks 9/10 = chunk 4 with s1/s0 rows zeroed.
        xh = xp.reshape(NPAIR, NKC, P, D).transpose(0, 2, 1, 3)
        ones = np.ones((NPAIR, P, NKC, 1), dtype=bf)
        xh = np.concatenate([ones, xh], axis=3)            # [pr, p, kc, DX]
        x4a = xh[:, :, 4:5, :].copy()
        x4a[:, 64:, 0, :] = 0                               # s0 view: zero s1 rows
        x4b = xh[:, :, 4:5, :].copy()
        x4b[:, :64, 0, :] = 0                               # s1 view: zero s0 rows
        xh = np.ascontiguousarray(np.concatenate([xh, x4a, x4b], axis=2))
        # transposed x (fp8, for mm1 weights): [pr, p(d in chunk), kg, dc, ks]
        xp8 = x_f8[c * BPC:(c + 1) * BPC].reshape(NPAIR, 2 * NQ, D)
        xTh = np.ascontiguousarray(
            xp8.reshape(NPAIR, KG, KGW, NDC, P).transpose(0, 4, 1, 3, 2))
        in_maps.append({"xs": xh, "xsT": xTh, "qT": qTh, "bT": bTh})
    return in_maps


def kernel(x, query, bias):
    from concourse.bass_utils import run_bass_kernel_spmd

    nc = build_program()
    in_maps = make_in_maps(np.asarray(x), np.asarray(query), np.asarray(bias))
    res = run_bass_kernel_spmd(nc, in_maps, core_ids=list(range(NCORES)))
    return np.concatenate(
        [r["out"].astype(np.float32) for r in res.results], axis=0)


if __name__ == "__main__":
    rng = np.random.default_rng(0)
    x = rng.standard_normal((B, NQ, D), dtype=np.float32)
    q = rng.standard_normal((NQ, D), dtype=np.float32) / 32.0
    bias = 0.01 * rng.standard_normal((NQ, NQ), dtype=np.float32)
    o = kernel(x, q, bias)
    print(o.shape, o.dtype)


# revision 33
# speedup vs baseline: 1.0194x; 1.0039x over previous
"""Trainium2 Bass kernel for ColumnAttention:
    out = softmax(query @ x^T + bias) @ x        (per batch sample)

Shapes: x [64, 576, 1024] f32, query [576, 1024] f32, bias [576, 576] f32.
Data-parallel over batch across 8 NeuronCores (8 samples per core).

Per-core program; samples processed in PAIRS (pair key axis 2*576 = 1152 =
9*128 so every mm1 k-chunk has full 128 partitions).

  mm1 (fp8 e4m3, DoubleRow):
        scoresT[k, q] = sum_d x[k, d] * qT[d, q]
        lhsT = host-pretransposed x, rhs = qT; DoubleRow packs two d-chunks
        per matmul (K=256 effective) for 2x PE throughput. q split 288+288
        into the two banks of one 2-bank PSUM tile. One strided DVE add
        applies the bias on drain; ACT exp writes bf16 attnT.
        (mm1 in e4m3 costs ~1.2e-2 max rel err vs the 2e-2 budget --
        measured bit-exact against a host fp8 simulation.)
  mm2 (bf16):
        out[q, d] = attnT[k, q]^T @ x'[k, d'] per sample, where x' has a
        leading all-ones column. d' split into 3 passes (343+341+341 cols)
        so each pass fits one PSUM bank; pass 0's output column 0 is then
        exactly the softmax denominator -- no extra matmuls for it.
        DVE reciprocal + ACT/DVE scale drains produce bf16 output tiles.
        The q=512:576 tails of BOTH samples run as column-tiled concurrent
        matmuls (s0 -> out partitions 0:64, s1 -> 64:128).
  All HBM inputs ride the sync HWDGE ring in consumption order (per-ring
  FIFO gives head transfers full DMA bandwidth); outputs ride the gpsimd
  ring. ~10 wide warmup matmuls lift the PE HAM clock gate to 8/8 during
  the initial DMA wait. mm1 of pair p+1 interleaves into mm2 of pair p.
"""

import sys

if "/opt/trn_rl_repo" not in sys.path:
    sys.path.insert(0, "/opt/trn_rl_repo")

import numpy as np
import ml_dtypes
from contextlib import ExitStack

B, NQ, D = 64, 576, 1024
NCORES = 8
BPC = B // NCORES      # samples per core
NPAIR = BPC // 2       # sample pairs per core

P = 128
NKC = 2 * NQ // P      # 9 pair k-chunks
NDC = D // P           # 8 d chunks
KG = 3                 # xT DMA k-groups (384 pair-k each)
KGW = 2 * NQ // KG     # 384
QMAIN = 4              # full 128-row q chunks per sample (tail handled jointly)
DX = D + 1             # x natural width incl leading ones column
# mm2 d-passes over x' columns: (x'_offset, width). Pass 0 includes the
# ones column, so its out d-range is [0, 342); passes 1/2 pure x.
PASSES = [(0, 343, 0, 342), (343, 341, 342, 683), (684, 341, 683, 1024)]

_BUILD_CACHE = {}


def build_program():
    """Build + compile the per-core Bass program. Returns the Bacc object."""
    if "nc" in _BUILD_CACHE:
        return _BUILD_CACHE["nc"]

    import concourse.mybir as mybir
    import concourse.tile as tile
    from concourse import bacc

    bf16 = mybir.dt.bfloat16
    fp8 = mybir.dt.float8e4
    f32 = mybir.dt.float32
    AF = mybir.ActivationFunctionType
    DR = mybir.MatmulPerfMode.DoubleRow

    nc = bacc.Bacc(trn_type="TRN2", target_bir_lowering=False, debug=False)

    # x chunks 0..8 = pair-k natural; chunks 9/10 = chunk 4 with the s1/s0
    # rows zeroed, so the k-straddle runs as full-row matmuls (partial-row
    # LDWEIGHTS cannot use the background weight buffer and would expose
    # ~100ns per straddle matmul).
    xs = nc.dram_tensor("xs", [NPAIR, P, NKC + 2, DX], bf16, kind="ExternalInput")
    xsT = nc.dram_tensor("xsT", [NPAIR, P, KG, NDC, KGW], fp8, kind="ExternalInput")
    qT = nc.dram_tensor("qT", [P, NDC, NQ], fp8, kind="ExternalInput")
    bT = nc.dram_tensor("bT", [P, NKC, NQ], bf16, kind="ExternalInput")
    out = nc.dram_tensor("out", [BPC, NQ, D], bf16, kind="ExternalOutput")

    with tile.TileContext(nc) as tc, ExitStack() as ctx:
        statics = ctx.enter_context(tc.tile_pool(name="statics", bufs=1))
        xpool = ctx.enter_context(tc.tile_pool(name="xpool", bufs=2))
        xtpool = ctx.enter_context(tc.tile_pool(name="xtpool", bufs=2))
        scpool = ctx.enter_context(tc.tile_pool(name="scpool", bufs=3))
        atpool = ctx.enter_context(tc.tile_pool(name="atpool", bufs=2))
        ompool = ctx.enter_context(tc.tile_pool(name="ompool", bufs=2))
        otpool = ctx.enter_context(tc.tile_pool(name="otpool", bufs=2))
        rpool = ctx.enter_context(tc.tile_pool(name="rpool", bufs=4))
        junkpool = ctx.enter_context(tc.tile_pool(name="junk", bufs=1))
        # PSUM: 2*2 + 4*1 = 8 banks
        psAB = ctx.enter_context(tc.tile_pool(name="psAB", bufs=2, space="PSUM"))
        psO = ctx.enter_context(tc.tile_pool(name="psO", bufs=4, space="PSUM"))

        ones_sb = statics.tile([P, 1], bf16)
        nc.vector.memset(ones_sb, 1.0)
        garbage = junkpool.tile([P, 512], bf16)
        nc.vector.memset(garbage, 0.0)

        # ---- PE warmup: wide matmuls during the input DMA wait so the HAM
        # clock gate reaches 8/8 before the first real matmul (N=1 matmuls
        # leave the array ~idle and do NOT lift the gate). ----
        warm = psO.tile([P, 512], f32, tag="po")
        for _ in range(9):
            nc.tensor.matmul(warm[0:1, :], ones_sb, garbage, start=True, stop=True)
        junk = junkpool.tile([P, 1], f32)
        nc.vector.tensor_copy(junk[0:1, :], warm[0:1, 0:1])

        # ---- static params, all on the sync ring in consumption order.
        # qT and the first xT k-group are split per d-half so the first
        # matmul waits on ~0.5MB, not 2MB; bias follows, then bulk x. ----
        qT_sb = statics.tile([P, NDC, NQ], fp8)
        xT0_sb = xtpool.tile([P, KG, NDC, KGW], fp8, tag="xT")
        for h in range(2):
            nc.sync.dma_start(out=qT_sb[:, 4 * h:4 * h + 4, :],
                              in_=qT.ap()[:, 4 * h:4 * h + 4, :])
            nc.sync.dma_start(out=xT0_sb[:, 0, 4 * h:4 * h + 4, :],
                              in_=xsT.ap()[0, :, 0, 4 * h:4 * h + 4, :])
        bT_sb = statics.tile([P, NKC, NQ], bf16)
        nc.sync.dma_start(out=bT_sb[:, 0:3, :], in_=bT.ap()[:, 0:3, :])
        nc.sync.dma_start(out=bT_sb[:, 3:9, :], in_=bT.ap()[:, 3:9, :])

        def load_pair(pr, xT_sb=None, kg_start=0):
            """k-progressive xT then natural x, all on the sync ring."""
            if xT_sb is None:
                xT_sb = xtpool.tile([P, KG, NDC, KGW], fp8, tag="xT")
            for kg in range(kg_start, KG):
                nc.sync.dma_start(out=xT_sb[:, kg], in_=xsT.ap()[pr, :, kg])
            x_sb = xpool.tile([P, NKC + 2, DX], bf16, tag="x")
            nc.sync.dma_start(out=x_sb[:, 0:4, :], in_=xs.ap()[pr, :, 0:4, :])
            nc.sync.dma_start(out=x_sb[:, 9:11, :], in_=xs.ap()[pr, :, 9:11, :])
            nc.sync.dma_start(out=x_sb[:, 5:9, :], in_=xs.ap()[pr, :, 5:9, :])
            return x_sb, xT_sb

        def mm1_chunk(xT_sb, attnT, kc):
            """One pair k-chunk of scoresT + bias + exp. fp8 DoubleRow packs
            two d-chunks per matmul (K=256 effective). q halves 288+288 live
            in the two banks of one 2-bank PSUM tile."""
            kg, ks = kc // 3, (kc % 3) * P
            pa = psAB.tile([P, 2, 512], f32, tag="pa")
            for dr in range(NDC // 2):
                w = xT_sb[:, kg, 2 * dr:2 * dr + 2, ks:ks + P]
                st, sp = dr == 0, dr == NDC // 2 - 1
                nc.tensor.matmul(pa[:, 0, 0:288], w, qT_sb[:, 2 * dr:2 * dr + 2, 0:288],
                                 start=st, stop=sp, perf_mode=DR)
                nc.tensor.matmul(pa[:, 1, 0:288], w, qT_sb[:, 2 * dr:2 * dr + 2, 288:576],
                                 start=st, stop=sp, perf_mode=DR)
            sc = scpool.tile([P, 2, 288], f32, tag="sc")
            bv = bT_sb[:, kc, :].rearrange("p (h q) -> p h q", h=2)
            nc.vector.tensor_add(sc, pa[:, :, 0:288], bv)
            av = attnT[:, kc, :].rearrange("p (h q) -> p h q", h=2)
            nc.scalar.activation(av, sc, AF.Exp)

        # slot = (attnT k-chunk, x_sb chunk): the straddle chunk 4 reads the
        # per-sample zero-padded x copy (chunks 9/10) with FULL 128-row
        # weights -- the other sample's attn rows hit zeroed x rows.
        S0_SLOTS = [(c, c) for c in range(4)] + [(4, 9)]
        S1_SLOTS = [(4, 10)] + [(c, c) for c in range(5, 9)]

        def mm2_main(pr, s, qc, ps, x_sb, attnT, o_main, r_):
            """One (sample, 128-row q-chunk, d-pass) of out = attn @ x'.
            Pass 0's column 0 is the softmax denominator."""
            off, w_, d0, d1 = PASSES[ps]
            qb = qc * P
            slots = S0_SLOTS if s == 0 else S1_SLOTS
            po = psO.tile([P, 512], f32, tag="po")
            for j, (c, xc) in enumerate(slots):
                wt = attnT[:, c, qb:qb + P]
                st, sp = j == 0, j == len(slots) - 1
                nc.tensor.matmul(po[:, 0:w_], wt, x_sb[:, xc, off:off + w_],
                                 start=st, stop=sp)
            if ps == 0:
                nc.vector.reciprocal(r_[:, :], po[:, 0:1])
                nc.scalar.activation(o_main[:, qc, d0:d1], po[:, 1:w_], AF.Copy,
                                     scale=r_[:, :])
            elif ps == 1:
                nc.vector.tensor_scalar_mul(o_main[:, qc, d0:d1], po[:, 0:w_], r_[:, :])
            else:
                nc.scalar.activation(o_main[:, qc, d0:d1], po[:, 0:w_], AF.Copy,
                                     scale=r_[:, :])

        def mm2_tail(pr, ps, x_sb, attnT, o_tail, r_):
            """q 512:576 of BOTH samples, column-tiled: s0 -> out partitions
            0:64, s1 -> 64:128, alternating so the half-array matmuls run
            concurrently."""
            off, w_, d0, d1 = PASSES[ps]
            po = psO.tile([P, 512], f32, tag="po")
            na, nb = len(S0_SLOTS), len(S1_SLOTS)
            for j in range(na + nb):
                s, (c, xc) = (0, S0_SLOTS[j // 2]) if j % 2 == 0 else (1, S1_SLOTS[j // 2])
                wt = attnT[:, c, 512:576]
                st = j < 2
                sp = j >= na + nb - 2
                nc.tensor.matmul(po[64 * s:64 * s + 64, 0:w_], wt,
                                 x_sb[:, xc, off:off + w_],
                                 start=st, stop=sp)
            if ps == 0:
                nc.vector.reciprocal(r_[:, :], po[:, 0:1])
                nc.scalar.activation(o_tail[:, d0:d1], po[:, 1:w_], AF.Copy,
                                     scale=r_[:, :])
            elif ps == 1:
                nc.vector.tensor_scalar_mul(o_tail[:, d0:d1], po[:, 0:w_], r_[:, :])
            else:
                nc.scalar.activation(o_tail[:, d0:d1], po[:, 0:w_], AF.Copy,
                                     scale=r_[:, :])

        # ---- prologue: pair 0 loads + mm1 (kg0 already in flight above).
        # The early chunks are DMA-paced with ~1-2us stalls; a couple of
        # garbage matmuls after each keep the HAM activity window busy so
        # the PE clock stays at 8/8 through the fill phase. ----
        x_cur, xT_cur = load_pair(0, xT_sb=xT0_sb, kg_start=1)
        attnT_cur = atpool.tile([P, NKC, NQ], bf16, tag="attnT")
        for kc in range(NKC):
            mm1_chunk(xT_cur, attnT_cur, kc)
            if kc < 6:
                for _ in range(2):
                    nc.tensor.matmul(warm[0:1, :], ones_sb, garbage,
                                     start=True, stop=True)
        nc.vector.tensor_copy(junk[0:1, :], warm[0:1, 1:2])

        # ---- steady: mm2(pair p) interleaved with mm1(pair p+1) ----
        for pr in range(NPAIR):
            if pr + 1 < NPAIR:
                x_nxt, xT_nxt = load_pair(pr + 1)
                attnT_nxt = atpool.tile([P, NKC, NQ], bf16, tag="attnT")
            else:
                x_nxt = xT_nxt = attnT_nxt = None

            o_mains = [ompool.tile([P, QMAIN, D], bf16, tag="om", name=f"om{pr}_{i}")
                       for i in range(2)]
            o_tail = otpool.tile([P, D], bf16, tag="ot")
            nunit = 0
            # mm1(p+1) chunks ride between mm2 units in PAIRS: each
            # fp8<->bf16 mode switch on the PE costs ~250ns, so fewer,
            # larger mm1 bursts beat one chunk per unit.
            CHUNK_AT = {3: (0, 1, 2), 7: (3, 4, 5), 11: (6, 7, 8)}

            def tick():
                nonlocal nunit
                if attnT_nxt is not None and nunit in CHUNK_AT:
                    for kc in CHUNK_AT[nunit]:
                        mm1_chunk(xT_nxt, attnT_nxt, kc)
                nunit += 1

            # On the last pair, stream the final chunks out per d-pass on the
            # otherwise-idle sync ring so the kernel tail has almost no
            # output left to drain after the final matmul.
            last = attnT_nxt is None
            for qc in range(QMAIN):
                rs = [rpool.tile([P, 1], f32, tag="r", name=f"r{pr}_{qc}_{i}")
                      for i in range(2)]
                for ps in range(3):
                    for s in range(2):
                        mm2_main(pr, s, qc, ps, x_cur, attnT_cur, o_mains[s], rs[s])
                        if last and qc == QMAIN - 1:
                            d0, d1 = PASSES[ps][2], PASSES[ps][3]
                            nc.sync.dma_start(
                                out=out.ap()[2 * pr + s, qc * P:(qc + 1) * P, d0:d1],
                                in_=o_mains[s][:, qc, d0:d1])
                        elif ps == 2:
                            # (s, qc) fully drained -> stream this chunk out
                            nc.gpsimd.dma_start(
                                out=out.ap()[2 * pr + s, qc * P:(qc + 1) * P, :],
                                in_=o_mains[s][:, qc, :])
                        tick()

            r_ = rpool.tile([P, 1], f32, tag="r")
            for ps in range(3):
                mm2_tail(pr, ps, x_cur, attnT_cur, o_tail, r_)
                if last:
                    d0, d1 = PASSES[ps][2], PASSES[ps][3]
                    for s in range(2):
                        nc.sync.dma_start(
                            out=out.ap()[2 * pr + s, 512:576, d0:d1],
                            in_=o_tail[64 * s:64 * s + 64, d0:d1])
                tick()
            if not last:
                for s in range(2):
                    nc.gpsimd.dma_start(out=out.ap()[2 * pr + s, 512:576, :],
                                        in_=o_tail[64 * s:64 * s + 64, :])

            x_cur, xT_cur, attnT_cur = x_nxt, xT_nxt, attnT_nxt

    nc.compile()
    _BUILD_CACHE["nc"] = nc
    return nc


def make_in_maps(x, query, bias):
    bf = ml_dtypes.bfloat16
    fp8 = ml_dtypes.float8_e4m3
    x_bf = x.astype(bf)
    x_f8 = x.astype(fp8)
    qTh = np.ascontiguousarray(
        query.T.astype(fp8).reshape(NDC, P, NQ).transpose(1, 0, 2))
    bTpair = np.concatenate([bias.T.astype(bf)] * 2, axis=0)       # [1152, 576]
    bTh = np.ascontiguousarray(bTpair.reshape(NKC, P, NQ).transpose(1, 0, 2))
    in_maps = []
    for c in range(NCORES):
        xp = x_bf[c * BPC:(c + 1) * BPC].reshape(NPAIR, 2 * NQ, D)
        # natural x with leading ones column, pair-k on partitions:
        # [pr, p, kc, 1+d]; chunks 9/10 = chunk 4 with s1/s0 rows zeroed.
        xh = xp.reshape(NPAIR, NKC, P, D).transpose(0, 2, 1, 3)
        ones = np.ones((NPAIR, P, NKC, 1), dtype=bf)
        xh = np.concatenate([ones, xh], axis=3)            # [pr, p, kc, DX]
        x4a = xh[:, :, 4:5, :].copy()
        x4a[:, 64:, 0, :] = 0                               # s0 view: zero s1 rows
        x4b = xh[:, :, 4:5, :].copy()
        x4b[:, :64, 0, :] = 0                               # s1 view: zero s0 rows
        xh = np.ascontiguousarray(np.concatenate([xh, x4a, x4b], axis=2))
        # transposed x (fp8, for mm1 weights): [pr, p(d in chunk), kg, dc, ks]
        xp8 = x_f8[c * BPC:(c + 1) * BPC].reshape(NPAIR, 2 * NQ, D)
        xTh = np.ascontiguousarray(
            xp8.reshape(NPAIR, KG, KGW, NDC, P).transpose(0, 4, 1, 3, 2))
        in_maps.append({"xs": xh, "xsT": xTh, "qT": qTh, "bT": bTh})
    return in_maps


def kernel(x, query, bias):
    from concourse.bass_utils import run_bass_kernel_spmd

    nc = build_program()
    in_maps = make_in_maps(np.asarray(x), np.asarray(query), np.asarray(bias))
    res = run_bass_kernel_spmd(nc, in_maps, core_ids=list(range(NCORES)))
    return np.concatenate(
        [r["out"].astype(np.float32) for r in res.results], axis=0)


if __name__ == "__main__":
    rng = np.random.default_rng(0)
    x = rng.standard_normal((B, NQ, D), dtype=np.float32)
    q = rng.standard_normal((NQ, D), dtype=np.float32) / 32.0
    bias = 0.01 * rng.standard_normal((NQ, NQ), dtype=np.float32)
    o = kernel(x, q, bias)
    print(o.shape, o.dtype)
